# revision 1
# baseline (speedup 1.0000x reference)
"""DenseSwAVCriterion loss on 8 Trainium2 NeuronCores.

Sharding: one (view, image) pair per core (M*N = 2*4 = 8 = n_cores).
Each core:
  - scans its image's scores [3136, 3000] for per-point row-max (memory-bound part)
  - computes per-region argmax (first index), gathers representative rows
    (indirect DMA), computes the pairwise-distance coherence loss locally
  - AllGathers the 8 [64, 3000] representative-score blocks (+ its local
    coherence loss) and redundantly computes the Sinkhorn/SwAV tail, which is
    formulated as 6 mat-vecs via the diagonal-scaling form of Sinkhorn
    (validated vs reference at ~6e-8 rel err).
Output: total loss scalar (core 0's copy is returned).
"""

import sys

import numpy as np

try:
    import concourse.bass  # noqa: F401
except ImportError:  # pragma: no cover
    sys.path.insert(0, "/opt/trn_rl_repo")
    sys.path.insert(0, "/root/.axon_site/_ro/trn_rl_repo")

M, N, P, D, K, R = 2, 4, 3136, 128, 3000, 64
NCORES = 8
TEMPERATURE = 0.1
EPSILON = 0.05
PDIST_EPS = 1e-6
BIG = 30000.0

STAGES = ["scan", "bcast", "seg1", "seg2", "seg", "gather", "local", "full", "tailsim"]

_cache = {}


def _build(stage="full"):
    import concourse.bacc as bacc
    import concourse.bass as bass
    import concourse.tile as tile
    from concourse import bass_isa, mybir
    from concourse.masks import make_identity

    lvl = STAGES.index(stage)
    tailsim = stage == "tailsim"
    if tailsim:
        lvl = -1  # skip all local phases; tail reads cc inputs directly

    f32 = mybir.dt.float32
    i32 = mybir.dt.int32
    X = mybir.AxisListType.X
    Alu = mybir.AluOpType
    Act = mybir.ActivationFunctionType
    f32r = mybir.dt.float32r

    def rr(ap):
        # fp32r view: 4x faster PE matmul, near-fp32 accuracy
        return ap.bitcast(f32r)
    nc = bacc.Bacc(
        "TRN2", target_bir_lowering=False, debug=False, num_devices=NCORES
    )

    sc = nc.dram_tensor("sc", [P, K], f32, kind="ExternalInput").ap()
    pj = nc.dram_tensor("pj", [P, D], f32, kind="ExternalInput").ap()
    pjT = nc.dram_tensor("pjT", [D, P], f32, kind="ExternalInput").ap()
    oh_d = nc.dram_tensor("oh", [R, P], f32, kind="ExternalInput").ap()
    invc_d = nc.dram_tensor("invc", [R, 1], f32, kind="ExternalInput").ap()
    out_d = nc.dram_tensor("out", [1, 1], f32, kind="ExternalOutput").ap()
    cc_out_in = lout_in = None
    if tailsim:
        cc_out_in = nc.dram_tensor(
            "cc_out_in", [NCORES * R, K], f32, kind="ExternalInput"
        ).ap()
        lout_in = nc.dram_tensor(
            "lout_in", [1, 8], f32, kind="ExternalInput"
        ).ap()

    NROW = 512          # rows per scan DMA (4 partition-tiles)
    NFULL = P // NROW   # 6 full chunks
    TAIL = P - NFULL * NROW  # 64
    NT = (P + 127) // 128  # 25 column-tiles of smax

    with tile.TileContext(nc) as tc:
        with (
            tc.tile_pool(name="const", bufs=1) as const,
            tc.tile_pool(name="small", bufs=1) as small,
            tc.tile_pool(name="psbig", bufs=1, space="PSUM") as psbig,
            tc.tile_pool(name="pssmall", bufs=1, space="PSUM") as pssmall,
            tc.tile_pool(name="dram", bufs=1, space="DRAM") as dram,
        ):
            # ---- tiny constants ----
            invc = const.tile([R, 1], f32)
            nc.sync.dma_start(out=invc[:], in_=invc_d[:])
            ident0 = const.tile([128, 128], f32)
            make_identity(nc, ident0[:])
            # DVE-copy so PE transposes reading (ident, smax_pt/DVE) collapse
            # their waits onto one engine.
            ident = const.tile([128, 128], f32)
            nc.vector.tensor_copy(ident[:], ident0[:])
            ones_col = const.tile([128, 1], f32)
            nc.vector.memset(ones_col[:], 1.0)
            ones_row = const.tile([1, 128], f32)
            nc.vector.memset(ones_row[:], 1.0)
            zeros_col = const.tile([128, 1], f32)
            nc.vector.memset(zeros_col[:], 0.0)
            eps_col = const.tile([128, 1], f32)
            nc.vector.memset(eps_col[:], PDIST_EPS)
            smax_pt = const.tile([128, NT], f32)

            # ---- stage A: row-max scan over scores (memory-bound) ----
            with tc.tile_pool(name="scan", bufs=2) as scan:
                sc_r = sc[0 : NFULL * NROW, :].rearrange(
                    "(j a p) k -> j p a k", a=NROW // 128, p=128
                )
                H1, H2 = K // 2, K // 4  # 1500, 750
                for j in range(NFULL):
                    t = scan.tile([128, NROW // 128, K], f32, tag="sct")
                    nc.sync.dma_start(out=t[:], in_=sc_r[j])
                    # in-place max-halving: DVE tensor_max runs ~2x faster
                    # than tensor_reduce, so shrink 3000 -> 750 first
                    nc.vector.tensor_max(
                        t[:, :, :H1], t[:, :, :H1], t[:, :, H1 : 2 * H1]
                    )
                    nc.vector.tensor_max(
                        t[:, :, :H2], t[:, :, :H2], t[:, :, H2 : 2 * H2]
                    )
                    nc.vector.reduce_max(
                        smax_pt[:, 4 * j : 4 * j + 4], t[:, :, :H2], axis=X
                    )
                t = scan.tile([128, 1, K], f32, tag="sct")
                nc.sync.dma_start(out=t[:TAIL, 0, :], in_=sc[NFULL * NROW :, :])
                nc.vector.tensor_max(
                    t[:TAIL, :, :H1], t[:TAIL, :, :H1], t[:TAIL, :, H1 : 2 * H1]
                )
                nc.vector.tensor_max(
                    t[:TAIL, :, :H2], t[:TAIL, :, :H2], t[:TAIL, :, H2 : 2 * H2]
                )
                nc.vector.reduce_max(
                    smax_pt[:TAIL, NT - 1 : NT], t[:TAIL, :, :H2], axis=X
                )
            if lvl == 0:
                nc.gpsimd.dma_start(out=out_d[:], in_=smax_pt[0:1, 0:1])

            # ============ local per-image phase (pools close before tail) ====
            with tc.tile_pool(name="mid", bufs=1) as mid:
                if lvl >= 1:
                    oh = mid.tile([R, P], f32)
                    nc.sync.dma_start(out=oh[:], in_=oh_d[:])
                    pjT_sb = mid.tile([D, P], f32)
                    nc.sync.dma_start(out=pjT_sb[:], in_=pjT[:])
                    imp = mid.tile([R, P], f32)  # iota - P along free axis
                    nc.gpsimd.iota(
                        imp[:], pattern=[[1, P]], base=-P, channel_multiplier=0,
                        allow_small_or_imprecise_dtypes=True,
                    )
                    # (2*oh-1)*BIG : +BIG in-region, -BIG off-region, so
                    # min(smax_b, ohmB) masks without losing exactness
                    ohmB = mid.tile([R, P], f32)
                    nc.vector.tensor_scalar(
                        ohmB[:], oh[:], 2.0 * BIG, -BIG, Alu.mult, Alu.add
                    )

                    # -- stage B: smax [128,25] -> row [1,P] -> bcast [64,P] --
                    row_ps = psbig.tile([1, P], f32, tag="big")
                    for t in range(NT):
                        w = 128 if t < NT - 1 else TAIL
                        nc.tensor.transpose(
                            out=row_ps[0:1, 128 * t : 128 * t + w],
                            in_=smax_pt[:w, t : t + 1],
                            identity=ident[:w, :w],
                        )
                    smax_row = mid.tile([1, P], f32)
                    nc.scalar.copy(smax_row[:], row_ps[:])
                    smax_b = mid.tile([R, P], f32)
                    nc.gpsimd.partition_broadcast(
                        smax_b[:], smax_row[:], channels=R
                    )
                    if lvl == 1:
                        nc.gpsimd.dma_start(out=out_d[:], in_=smax_b[0:1, 0:1])

                if lvl >= 2:
                    # ---- stage C: segment max + first-index argmax ----
                    # masked = min(smax_b, ohmB): in-region exact smax,
                    # off-region -BIG (never equal to segmax)
                    masked = mid.tile([R, P], f32, tag="scr2")
                    nc.vector.tensor_tensor(
                        masked[:], smax_b[:], ohmB[:], Alu.min
                    )
                    segmax = small.tile([R, 1], f32)
                    nc.vector.reduce_max(segmax[:], masked[:], axis=X)
                    if lvl == 2:
                        nc.gpsimd.dma_start(out=out_d[:], in_=segmax[0:1, 0:1])
                    # cand = (masked == segmax) * (iota - P), fused stt
                    candp = mid.tile([R, P], f32, tag="scr")
                    nc.vector.scalar_tensor_tensor(
                        candp[:], masked[:], segmax[:], imp[:],
                        Alu.is_equal, Alu.mult,
                    )
                    if lvl == 3:
                        nc.gpsimd.dma_start(out=out_d[:], in_=candp[0:1, 0:1])
                    candmin = small.tile([R, 1], f32)
                    nc.vector.tensor_reduce(
                        candmin[:], candp[:], axis=X, op=Alu.min
                    )
                    rep_i = small.tile([R, 1], i32)
                    nc.vector.tensor_scalar(
                        rep_i[:], candmin[:], float(P), None, Alu.add
                    )
                    if lvl == 4:
                        nc.gpsimd.dma_start(out=out_d[:], in_=candmin[0:1, 0:1])

                if lvl >= 5:
                    # ---- stage D: representative-row gathers ----
                    # rep_sc via indirect DMA (scores too big for SBUF);
                    # issued first so it overlaps all of stage E below.
                    rep_sc = mid.tile([R, K], f32)
                    nc.gpsimd.indirect_dma_start(
                        out=rep_sc[:], out_offset=None, in_=sc[:],
                        in_offset=bass.IndirectOffsetOnAxis(
                            ap=rep_i[:, :1], axis=0
                        ),
                    )
                    # rep_pj via one-hot matmul from the SBUF-resident projs:
                    # sel[r,q] = (q == rep_idx[r]);  rep_pj = sel @ pj
                    pj_sb = mid.tile([128, NT, D], f32)  # [p, chunk, d]
                    nc.sync.dma_start(
                        out=pj_sb[:, : NT - 1, :],
                        in_=pj[: (NT - 1) * 128, :].rearrange(
                            "(t p) d -> p t d", p=128
                        ),
                    )
                    nc.sync.dma_start(
                        out=pj_sb[:TAIL, NT - 1, :],
                        in_=pj[(NT - 1) * 128 :, :],
                    )
                    sel = mid.tile([R, P], f32, tag="scr")
                    nc.vector.tensor_scalar(
                        sel[:], imp[:], candmin[:], None, Alu.is_equal
                    )
                    selt_ps = psbig.tile([128, NT, R], f32, tag="big")
                    for tt in range(NT):
                        w = 128 if tt < NT - 1 else TAIL
                        nc.tensor.transpose(
                            out=selt_ps[:w, tt, :],
                            in_=sel[:, 128 * tt : 128 * tt + w],
                            identity=ident[:R, :R],
                        )
                    selt = mid.tile([128, NT, R], f32)
                    nc.vector.tensor_copy(selt[:], selt_ps[:])
                    rep_pj_ps = pssmall.tile([R, D], f32, tag="sm")
                    for tt in range(NT):
                        w = 128 if tt < NT - 1 else TAIL
                        nc.tensor.matmul(
                            out=rep_pj_ps[:],
                            lhsT=selt[:w, tt, :],
                            rhs=pj_sb[:w, tt, :],
                            start=(tt == 0), stop=(tt == NT - 1),
                        )
                    rep_pj = small.tile([R, D], f32)
                    nc.vector.tensor_copy(rep_pj[:], rep_pj_ps[:])
                    if lvl == 5:
                        nc.gpsimd.dma_start(out=out_d[:], in_=rep_pj[0:1, 0:1])

                if lvl >= 6:
                    # ---- stage E: pairwise-distance coherence loss ----
                    gT_ps = psbig.tile([D, P], f32, tag="big")
                    for c0 in range(0, P, 512):
                        c1 = min(c0 + 512, P)
                        nc.tensor.matmul(
                            out=gT_ps[:, c0:c1], lhsT=rep_pj[:],
                            rhs=oh[:, c0:c1], start=True, stop=True,
                        )
                    diff = mid.tile([D, P], f32, tag="scr")
                    nc.vector.tensor_sub(diff[:], pjT_sb[:], gT_ps[:])
                    sq = mid.tile([D, P], f32, tag="scr2")
                    nc.scalar.activation(
                        sq[:], diff[:], Act.Square, bias=eps_col[:], scale=1.0
                    )
                    d2_ps = psbig.tile([1, P], f32, tag="big")
                    for c0 in range(0, P, 512):
                        c1 = min(c0 + 512, P)
                        nc.tensor.matmul(
                            out=d2_ps[0:1, c0:c1], lhsT=ones_col[:],
                            rhs=sq[:, c0:c1], start=True, stop=True,
                        )
                    d_row = mid.tile([1, P], f32)
                    nc.scalar.activation(
                        d_row[:], d2_ps[:], Act.Sqrt, bias=zeros_col[:1, :]
                    )
                    d_b = mid.tile([R, P], f32, tag="scr")
                    nc.gpsimd.partition_broadcast(d_b[:], d_row[:], channels=R)
                    dscr = mid.tile([R, P], f32, tag="scr2")
                    nc.vector.tensor_mul(dscr[:], oh[:], d_b[:])
                    dsink = mid.tile([R, P], f32, tag="scr")
                    regsum = small.tile([R, 1], f32)
                    nc.scalar.activation(
                        dsink[:], dscr[:], Act.Copy, accum_out=regsum[:]
                    )
                    regmean = small.tile([R, 1], f32)
                    nc.vector.tensor_mul(regmean[:], regsum[:], invc[:])
                    limg_ps = pssmall.tile([1, 1], f32, tag="sm")
                    nc.tensor.matmul(
                        out=limg_ps[:], lhsT=regmean[:], rhs=ones_col[:R, :],
                        start=True, stop=True,
                    )
                    limg = small.tile([1, 1], f32)
                    nc.scalar.activation(
                        limg[:], limg_ps[:], Act.Copy, scale=1.0 / R
                    )
                    if lvl == 6:
                        nc.gpsimd.dma_start(out=out_d[:], in_=limg[:])

                if lvl >= 7:
                    # ---- stage F: exchange rep scores; losses via AllReduce
                    # (decoupled so the pdist chain is off the AG's path) ----
                    cc_in = dram.tile([R, K], f32)
                    nc.sync.dma_start(out=cc_in[:], in_=rep_sc[:])
                    lin_sb = small.tile([1, 8], f32)
                    nc.vector.memset(lin_sb[:], 0.0)
                    nc.vector.tensor_copy(lin_sb[0:1, 0:1], limg[:])
                    lin = dram.tile([1, 8], f32)
                    nc.sync.dma_start(out=lin[:], in_=lin_sb[:])

            if lvl >= 7 or tailsim:
              if tailsim:
                cc_out = cc_out_in
                lsum = small.tile([1, 1], f32)
                nc.sync.dma_start(out=lsum[:], in_=lout_in[0:1, 0:1])
              else:
                cc_out = dram.tile([NCORES * R, K], f32, addr_space="Shared")
                nc.gpsimd.collective_compute(
                    "AllGather",
                    mybir.AluOpType.bypass,
                    replica_groups=[list(range(NCORES))],
                    ins=[cc_in[:]],
                    outs=[cc_out[:]],
                ) if not tailsim else None
                if not tailsim:
                    lout = dram.tile([1, 8], f32, addr_space="Shared")
                    nc.gpsimd.collective_compute(
                        "AllReduce",
                        mybir.AluOpType.add,
                        replica_groups=[list(range(NCORES))],
                        ins=[lin[:]],
                        outs=[lout[:]],
                    )
                    lsum = small.tile([1, 1], f32)
                    nc.sync.dma_start(out=lsum[:], in_=lout[0:1, 0:1])

              # ============ replicated SwAV tail ============
              with tc.tile_pool(name="tail", bufs=1) as tl:
                  v0 = tl.tile([128, 2, K], f32)
                  nc.sync.dma_start(
                      out=v0[:],
                      in_=cc_out[0:256, :].rearrange("(g p) k -> p g k", p=128),
                  )
                  v1 = tl.tile([128, 2, K], f32)
                  nc.sync.dma_start(
                      out=v1[:],
                      in_=cc_out[256:512, :].rearrange("(g p) k -> p g k", p=128),
                  )
                  v0a, v0b = v0[:, 0, :], v0[:, 1, :]
                  v1a, v1b = v1[:, 0, :], v1[:, 1, :]

                  # global max of v0/eps; exp fused with the 1/eps scaling
                  vmx = tl.tile([128, K], f32, tag="tscr")
                  nc.vector.tensor_max(vmx[:], v0a, v0b)
                  nc.vector.tensor_max(
                      vmx[:, : K // 2], vmx[:, : K // 2], vmx[:, K // 2 :]
                  )
                  mxc = small.tile([128, 1], f32)
                  nc.vector.reduce_max(mxc[:], vmx[:, : K // 2], axis=X)
                  gm = small.tile([128, 1], f32)
                  nc.gpsimd.partition_all_reduce(
                      gm[:], mxc[:], channels=128,
                      reduce_op=bass_isa.ReduceOp.max,
                  )
                  gmneg = small.tile([128, 1], f32)
                  nc.vector.tensor_scalar_mul(gmneg[:], gm[:], -1.0 / EPSILON)
                  q0a = tl.tile([128, K], f32)
                  nc.scalar.activation(
                      rr(q0a[:]), v0a, Act.Exp, bias=gmneg[:],
                      scale=1.0 / EPSILON,
                  )
                  q0b = tl.tile([128, K], f32)
                  nc.scalar.activation(
                      rr(q0b[:]), v0b, Act.Exp, bias=gmneg[:],
                      scale=1.0 / EPSILON,
                  )

                  # Sinkhorn, scale-invariant diagonal form:
                  #   mhat_u[k] = sum_n Q[n,k]*bhat[n] (PE matmul, lhsT=bhat)
                  #   ahat = bcast(recip(mhat_u))
                  #   mhat_c[n] = sum_k Q[n,k]*ahat[k] (DVE ttr)
                  #   bhat = recip(mhat_c)
                  bh_a, bh_b = None, None
                  ahat_b = None
                  mca = mcb = None
                  for it in range(3):
                      mu_ps = psbig.tile([1, K], f32, tag="big")
                      for c0 in range(0, K, 500):
                          c1 = c0 + 500
                          nc.tensor.matmul(
                              out=mu_ps[0:1, c0:c1],
                              lhsT=rr(ones_col[:] if it == 0 else bh_a[:]),
                              rhs=rr(q0a[:, c0:c1]), start=True, stop=False,
                          )
                          nc.tensor.matmul(
                              out=mu_ps[0:1, c0:c1],
                              lhsT=rr(ones_col[:] if it == 0 else bh_b[:]),
                              rhs=rr(q0b[:, c0:c1]), start=False, stop=True,
                          )
                      recip_row = small.tile([1, K], f32, tag="mu_row")
                      with nc.allow_low_precision(reason="fp32r view"):
                          nc.vector.reciprocal(rr(recip_row[:]), mu_ps[:])
                      ahat_b = psbig.tile([128, K], f32, tag="big")
                      for c0 in range(0, K, 500):
                          c1 = c0 + 500
                          nc.tensor.matmul(
                              out=ahat_b[:, c0:c1], lhsT=rr(ones_row[:]),
                              rhs=rr(recip_row[0:1, c0:c1]),
                              start=True, stop=True,
                          )
                      tscr = tl.tile([128, K], f32, tag="tscr")
                      nc.vector.tensor_mul(tscr[:], q0a[:], ahat_b[:])
                      tsink = tl.tile([128, K], f32, tag="tsink")
                      mca = small.tile([128, 1], f32, tag="mca")
                      nc.scalar.activation(
                          tsink[:], tscr[:], Act.Copy, accum_out=mca[:]
                      )
                      tscr2 = tl.tile([128, K], f32, tag="tscr")
                      nc.vector.tensor_mul(tscr2[:], q0b[:], ahat_b[:])
                      tsink2 = tl.tile([128, K], f32, tag="tsink")
                      mcb = small.tile([128, 1], f32, tag="mcb")
                      nc.scalar.activation(
                          tsink2[:], tscr2[:], Act.Copy, accum_out=mcb[:]
                      )
                      if it < 2:
                          bh_a = small.tile([128, 1], f32, tag="bha")
                          bh_b = small.tile([128, 1], f32, tag="bhb")
                          with nc.allow_low_precision(reason="fp32r view"):
                              nc.vector.reciprocal(rr(bh_a[:]), mca[:])
                              nc.vector.reciprocal(rr(bh_b[:]), mcb[:])

                  # CE prep that depends only on (q0, v1): overlaps the
                  # sinkhorn iterations below.
                  Ws, logbetas = [], []
                  for idx, (q0, v1) in enumerate(((q0a, v1a), (q0b, v1b))):
                      W = tl.tile([128, K], f32, tag=f"W{idx}")
                      nc.vector.tensor_mul(W[:], q0[:], v1[:])
                      Ws.append(W)
                      m1 = small.tile([128, 1], f32, tag=f"m1{idx}")
                      nc.vector.reduce_max(m1[:], v1[:], axis=X)
                      m1n = small.tile([128, 1], f32, tag=f"m1n{idx}")
                      nc.vector.tensor_scalar_mul(
                          m1n[:], m1[:], -1.0 / TEMPERATURE
                      )
                      escr = tl.tile([128, K], f32, tag="escr")
                      Z = small.tile([128, 1], f32, tag=f"Z{idx}")
                      nc.scalar.activation(
                          escr[:], v1[:], Act.Exp, bias=m1n[:],
                          scale=1.0 / TEMPERATURE, accum_out=Z[:],
                      )
                      logZ = small.tile([128, 1], f32, tag=f"logZ{idx}")
                      nc.scalar.activation(
                          logZ[:], Z[:], Act.Ln, bias=zeros_col[:]
                      )
                      logbeta = small.tile([128, 1], f32, tag=f"logbeta{idx}")
                      nc.vector.tensor_scalar(
                          logbeta[:], m1[:], 1.0 / TEMPERATURE, None,
                          Alu.mult,
                      )
                      nc.vector.tensor_add(logbeta[:], logbeta[:], logZ[:])
                      logbetas.append(logbeta)

                  g_a = small.tile([128, 1], f32)
                  nc.vector.reciprocal(g_a[:], mca[:])
                  g_b = small.tile([128, 1], f32)
                  nc.vector.reciprocal(g_b[:], mcb[:])

                  # CE: p1' = sum_k W*ahat3 ; rowsum(A) = g*mca ;
                  # ce_n = p1'*g/T - logbeta*rowsum(A)
                  ce_parts = []
                  for idx, (g_, mc, W, logbeta) in enumerate(
                      ((g_a, mca, Ws[0], logbetas[0]),
                       (g_b, mcb, Ws[1], logbetas[1]))
                  ):
                      pscr = tl.tile([128, K], f32, tag="tscr")
                      nc.vector.tensor_mul(pscr[:], W[:], ahat_b[:])
                      psink = tl.tile([128, K], f32, tag="tsink")
                      p1 = small.tile([128, 1], f32, tag="p1")
                      nc.scalar.activation(
                          psink[:], pscr[:], Act.Copy, accum_out=p1[:]
                      )
                      rA = small.tile([128, 1], f32, tag="rA")
                      nc.vector.tensor_mul(rA[:], g_[:], mc[:])
                      t1 = small.tile([128, 1], f32, tag="t1")
                      nc.vector.tensor_mul(t1[:], logbeta[:], rA[:])
                      cpart = small.tile([128, 1], f32, tag=f"cpart{idx}")
                      nc.vector.tensor_mul(cpart[:], p1[:], g_[:])
                      nc.vector.tensor_scalar_mul(
                          cpart[:], cpart[:], 1.0 / TEMPERATURE
                      )
                      nc.vector.tensor_sub(cpart[:], cpart[:], t1[:])
                      ce_parts.append(cpart)

                  csum = small.tile([128, 1], f32)
                  nc.vector.tensor_add(csum[:], ce_parts[0][:], ce_parts[1][:])
                  # total = (mean limg + (-ce_sum/(N*R)))/2
                  ce_ps = pssmall.tile([1, 1], f32, tag="sm")
                  nc.tensor.matmul(
                      out=ce_ps[:], lhsT=csum[:], rhs=ones_col[:],
                      start=True, stop=True,
                  )
                  t_ce = small.tile([1, 1], f32)
                  nc.scalar.activation(
                      t_ce[:], ce_ps[:], Act.Copy,
                      scale=-1.0 / (N * R * (M - 1) * 2),
                  )
                  t_lcoh = small.tile([1, 1], f32)
                  nc.scalar.activation(
                      t_lcoh[:], lsum[:], Act.Copy, scale=1.0 / (NCORES * 2)
                  )
                  tot = small.tile([1, 1], f32)
                  nc.vector.tensor_add(tot[:], t_lcoh[:], t_ce[:])
                  nc.gpsimd.dma_start(out=out_d[:], in_=tot[:])

    nc.compile()
    return nc


def _get_nc():
    if "nc" not in _cache:
        _cache["nc"] = _build()
    return _cache["nc"]


last_results = None


def kernel(projs, scores, regions_idxs, _trace=False):
    from concourse import bass_utils

    projs = np.asarray(projs, dtype=np.float32)
    scores = np.asarray(scores, dtype=np.float32)
    regions = np.asarray(regions_idxs).astype(np.int32)

    in_maps = []
    rng = np.arange(R, dtype=np.int32)
    for c in range(NCORES):
        v, i = divmod(c, N)
        reg = regions[v, i]
        oh = (rng[:, None] == reg[None, :]).astype(np.float32)
        cnt = oh.sum(axis=1, dtype=np.float32)
        in_maps.append(
            {
                "sc": np.ascontiguousarray(scores[v, i]),
                "pj": np.ascontiguousarray(projs[v, i]),
                "pjT": np.ascontiguousarray(projs[v, i].T),
                "oh": oh,
                "invc": (1.0 / cnt).astype(np.float32).reshape(R, 1),
            }
        )

    nc = _get_nc()
    global last_results
    last_results = bass_utils.run_bass_kernel_spmd(
        nc, in_maps, core_ids=list(range(NCORES)), trace=_trace
    )
    out = last_results.results[0]["out"]
    return np.float32(out.reshape(())[()])


if __name__ == "__main__":
    rs = np.random.RandomState(0)
    demo = {
        "projs": rs.randn(M, N, P, D).astype(np.float32),
        "scores": rs.randn(M, N, P, K).astype(np.float32),
        "regions_idxs": rs.randint(0, R, size=(M, N, P)).astype(np.int64),
    }
    print("loss:", kernel(**demo))

